# revision 32
# baseline (speedup 1.0000x reference)
"""MultiHeadAttention + residual + LayerNorm, 8-core Trainium2 Bass kernel.

Problem (hardcoded, self-contained):
  q,k,v: (4, 2048, 1024) f32; zero_mask: (4,1,1,2048) f32 (zeros per spec);
  Wq/Wk/Wv/Wo: (1024, 1024) f32; gamma/beta: (1024,) f32.
  out = LayerNorm(softmax(qh @ kh^T / 8 + mask*-1e9) @ vh @ Wo.T + q)

Sharding: pure token/data parallel, zero collectives. Core c handles
batch b=c//2, query rows [(c%2)*1024, (c%2+1)*1024). Each core computes
full K/V projections for its batch, attention + output projection +
residual + LayerNorm for its own 1024 query tokens.

Fast path (zero mask, unit gamma, zero beta — the spec's fill):
  Everything in fp8e4m3 with DoubleRow (K=256) matmuls at 0.5 cycles/row.
  softmax exp is Schraudolph-style: the S matmul's weights are pre-scaled
  so PSUM = 8*log2(e)*logits; adding 40 and converting f32->int8 (round+
  saturate, identical on ACT/DVE/GPSIMD) yields the bit pattern of
  exp(logits)*2^-2 in fp8e4m3, which the ctx matmul consumes via bitcast.
  The uniform 2^-2 factor cancels in softmax. The exp "pass" is thus a
  plain convert, round-robined across all three elementwise engines.
  Denominators come free as a ones-column in vh (PSUM row 64); ctx is
  normalized (and x32 rescaled for fp8) with reciprocal + partition
  broadcast + one scalar_tensor_tensor per head. Out-projection undoes
  the x32 weight scales (1/1024) inside the residual-add STT.

Fallback path (nonzero mask or non-trivial gamma/beta): the original
bf16 kernel, compiled lazily only if such inputs ever show up.
"""

import numpy as np

try:
    import concourse.bass as bass
except ImportError:  # fresh grading dir: repo is staged in the container
    import sys

    sys.path.insert(0, "/opt/trn_rl_repo")
    import concourse.bass as bass

import ml_dtypes
import concourse.tile as tile
from concourse import bacc, mybir
from concourse.bass_utils import run_bass_kernel_spmd

F32 = mybir.dt.float32
BF = mybir.dt.bfloat16
FP8 = mybir.dt.float8e4
FP8E5 = mybir.dt.float8e5
I8 = mybir.dt.int8
AF = mybir.ActivationFunctionType
OP = mybir.AluOpType
PM = mybir.MatmulPerfMode
BF_NP = ml_dtypes.bfloat16
E4_NP = ml_dtypes.float8_e4m3

BS, SEQ, D, H, DH = 4, 2048, 1024, 16, 64
NCORE = 8
TQ = 1024  # query tokens per core
P = 128
NJT = SEQ // P  # 16 key tiles
NG = NJT // 2  # 8 key-tile pairs
NIC = 2  # query chunks of 512
EPS = 1e-5
NEG = -1e9

LOG2E = float(np.log2(np.e))
# P is materialized as e5m2 bit patterns: bits = round(4*log2e*L + 48).
# The uniform 2^((48-62)/4) factor cancels in softmax; range of bits for
# |L| <= 8.1 sigma is [1, 95] — no NaN/inf encodings, no clamps needed.
BITS_BIAS = 48.0
W_SIDE_SCALE = float(np.sqrt(4.0 * np.log2(np.e) / 8.0))  # per-side q/k scale

# engine shares for the 256 exp-converts (ACT, DVE, POOL), tuned by profile
CONV_W = (0.49, 0.22, 0.29)


def bcast_pap(ap1d, p=P):
    """Partition-broadcast AP: [n] -> [p, n] with partition step 0."""
    return bass.AP(tensor=ap1d.tensor, offset=ap1d.offset, ap=[[0, p], *ap1d.ap])


def _conv_engine_seq(n):
    """Largest-remainder interleave of n units across 3 engines."""
    acc = [0.0, 0.0, 0.0]
    seq = []
    for _ in range(n):
        for i in range(3):
            acc[i] += CONV_W[i]
        i = max(range(3), key=lambda j: acc[j])
        acc[i] -= 1.0
        seq.append(i)
    return seq


def _build_fp8():
    nc = bacc.Bacc(None, target_bir_lowering=False)

    qT_d = nc.declare_dram_parameter("qT8", [P, 4, 2, TQ], FP8, isOutput=False)
    kT_d = nc.declare_dram_parameter("kT8", [P, 4, 2, SEQ], FP8, isOutput=False)
    vT_d = nc.declare_dram_parameter("vT8", [P, 4, 2, SEQ], FP8, isOutput=False)
    wq_d = nc.declare_dram_parameter("wq8", [P, 4, 2, D], FP8, isOutput=False)
    wk_d = nc.declare_dram_parameter("wk8", [P, 4, 2, D], FP8, isOutput=False)
    wv_d = nc.declare_dram_parameter("wv8", [P, 4, 2, D], FP8, isOutput=False)
    wo_d = nc.declare_dram_parameter("wo8", [P, 4, 2, D], FP8, isOutput=False)
    q_d = nc.declare_dram_parameter("q", [TQ, D], F32, isOutput=False)
    out_d = nc.declare_dram_parameter("out", [TQ, D], F32, isOutput=True)

    conv_seq = _conv_engine_seq(NIC * H * NG)
    conv_idx = [0]

    with tile.TileContext(nc) as tc:
        with (
            tc.tile_pool(name="wpool", bufs=1) as wpool,
            tc.tile_pool(name="xpool", bufs=1) as xpool,
            tc.tile_pool(name="hpool", bufs=1) as hpool,
            tc.tile_pool(name="ppool", bufs=14) as ppool,
            tc.tile_pool(name="npool", bufs=4) as npool,
            tc.tile_pool(name="opool", bufs=2) as opool,
            tc.tile_pool(name="spool", bufs=3, space="PSUM") as spool,
            tc.tile_pool(name="cpool", bufs=2, space="PSUM") as cpool,
        ):
            wq = wpool.tile([P, 4, 2, D], FP8, tag="wq")
            wk = wpool.tile([P, 4, 2, D], FP8, tag="wk")
            wv = wpool.tile([P, 4, 2, D], FP8, tag="wv")
            wo = wpool.tile([P, 4, 2, D], FP8, tag="wo")
            kT = xpool.tile([P, 4, 2, SEQ], FP8, tag="kT")
            vT = xpool.tile([P, 4, 2, SEQ], FP8, tag="vT")
            qT = xpool.tile([P, 4, 2, TQ], FP8, tag="qT")

            qhT = hpool.tile([P, 4, 2, TQ], FP8, tag="qhT")
            khT = hpool.tile([P, 4, 2, SEQ], FP8, tag="khT")
            # 65 cols per head (64 ch + ones for the Z row); H padded to 17
            # so each head's 128-wide ctx lhsT window stays in-bounds. The
            # window covers the next head's columns too — those land in psum
            # rows 65..127, which are never read.
            vh = hpool.tile([P, NJT, H + 1, DH + 1], FP8, tag="vh")
            ctx8 = hpool.tile([P, 4, 2, TQ], FP8, tag="ctx8")
            bias_ap = hpool.tile([P, 1], F32, tag="bias")
            nc.vector.memset(bias_ap, BITS_BIAS)

            # chunked loads ordered so the first K-proj matmul starts early
            nc.sync.dma_start(wk, wk_d[:, :, :, :])
            for ch in range(4):
                csl = slice(ch * 512, (ch + 1) * 512)
                nc.sync.dma_start(kT[:, :, :, csl], kT_d[:, :, :, csl])
            nc.sync.dma_start(wv, wv_d[:, :, :, :])
            for ch in range(4):
                csl = slice(ch * 512, (ch + 1) * 512)
                nc.sync.dma_start(vT[:, :, :, csl], vT_d[:, :, :, csl])
            nc.sync.dma_start(wq, wq_d[:, :, :, :])
            for ch in range(2):
                csl = slice(ch * 512, (ch + 1) * 512)
                nc.sync.dma_start(qT[:, :, :, csl], qT_d[:, :, :, csl])
            nc.sync.dma_start(wo, wo_d[:, :, :, :])

            # weighted round-robin of projection copies (ACT, DVE, POOL)
            copy_seq = []
            _acc = [0.0, 0.0, 0.0]
            _cw = (0.5, 0.2, 0.3)
            for _ in range(80):
                for i in range(3):
                    _acc[i] += _cw[i]
                i = max(range(3), key=lambda j: _acc[j])
                _acc[i] -= 1.0
                copy_seq.append(i)
            copy_rr = [0]

            def conv_copy(dst, src):
                e = copy_seq[copy_rr[0] % len(copy_seq)]
                copy_rr[0] += 1
                if e == 0:
                    nc.scalar.activation(dst, src, AF.Copy)
                elif e == 1:
                    nc.vector.tensor_copy(dst, src)
                else:
                    nc.gpsimd.tensor_copy(dst, src)

            def proj_psum(w, x, m, csl):
                """One [128, 512] projection psum: out-ch block m, tokens csl."""
                ps = spool.tile([P, 512], F32, tag="s", name="pj")
                for pr in range(4):
                    nc.tensor.matmul(
                        ps,
                        w[:, pr, :, m * P : (m + 1) * P],
                        x[:, pr, :, csl],
                        start=(pr == 0),
                        stop=(pr == 3),
                        perf_mode=PM.DoubleRow,
                    )
                return ps

            def qk_chunk(w, x, dst, ch):
                """Q/K projection for a 512-token chunk -> dst[:, a, kt, csl]."""
                csl = slice(ch * 512, (ch + 1) * 512)
                for m in range(8):
                    ps = proj_psum(w, x, m, csl)
                    conv_copy(dst[:, m // 2, m % 2, csl], ps)

            def vh_win(g, h):
                """[128, 2, 128] ctx lhsT window at (key-tile pair g, head h)."""
                t = vh[:, :, :, :]
                s1 = t.ap[1][0]  # jt stride
                s2 = t.ap[2][0]  # head stride
                return bass.AP(
                    tensor=t.tensor,
                    offset=t.offset + 2 * g * s1 + h * s2,
                    ap=[list(t.ap[0]), [s1, 2], [1, P]],
                )

            # ---- phase 1: K proj (all), V proj (all), Q chunk 0 ----
            for ch in range(4):
                qk_chunk(wk, kT, khT, ch)
            nc.vector.memset(vh[:, :, 0:H, DH : DH + 1], 1.0)
            nc.vector.memset(vh[:, :, H, :], 0.0)
            for jt in range(NJT):
                jsl = slice(jt * P, (jt + 1) * P)
                for hg in range(2):
                    ps = spool.tile([P, 512], F32, tag="s", name="vj")
                    for pr in range(4):
                        nc.tensor.matmul(
                            ps,
                            vT[:, pr, :, jsl],
                            wv[:, pr, :, hg * 512 : (hg + 1) * 512],
                            start=(pr == 0),
                            stop=(pr == 3),
                            perf_mode=PM.DoubleRow,
                        )
                    conv_copy(
                        vh[:, jt, 8 * hg : 8 * hg + 8, 0:DH],
                        ps.rearrange("p (h c) -> p h c", c=DH),
                    )
            qk_chunk(wq, qT, qhT, 0)

            # ---- phase 2/3: per query chunk: attention all heads, then
            # out-proj + LayerNorm for those tokens (interleaved) ----
            def s_convert(pt, ps2):
                # bits = max(S + 48, 0): negative bits would land in e5m2's
                # NaN/inf encodings; bits=0 is P=+0, correct for such keys
                e = conv_seq[conv_idx[0]]
                conv_idx[0] += 1
                if e == 0:
                    nc.scalar.activation(pt, ps2, AF.Relu, bias=bias_ap[:, 0:1])
                elif e == 1:
                    nc.vector.tensor_scalar(pt, ps2, BITS_BIAS, 0.0, OP.add, OP.max)
                else:
                    nc.gpsimd.tensor_scalar(pt, ps2, BITS_BIAS, 0.0, OP.add, OP.max)

            # one-head software pipeline: while head h's S matmuls + converts
            # are queued, the PREVIOUS head's ctx matmuls are interleaved one
            # per group (their convert deps are a full head old), and its
            # normalize is emitted at the end of the head.
            pend = [None]

            def emit_ctx_one(g):
                h, ic, pts, ct = pend[0]
                nc.tensor.matmul(
                    ct[:, :],
                    vh_win(g, h),
                    pts[g][:, :, :].bitcast(FP8E5),
                    start=(g == 0),
                    stop=(g == NG - 1),
                    perf_mode=PM.DoubleRow,
                )

            def emit_norm():
                # normalize: ctx8 = ct / Z  (Z = psum row 64 via ones column)
                h, ic, pts, ct = pend[0]
                pend[0] = None
                isl = slice(ic * 512, (ic + 1) * 512)
                rr = npool.tile([1, 512], F32, tag="rr")
                nc.vector.reciprocal(rr[0:1, :], ct[64:65, :])
                bc = npool.tile([64, 512], F32, tag="bc")
                nc.gpsimd.partition_broadcast(bc, rr[0:1, :])
                nc.vector.tensor_tensor(
                    ctx8[64 * (h % 2) : 64 * (h % 2) + 64, (h // 2) // 2,
                         (h // 2) % 2, isl],
                    ct[0:64, :],
                    bc[:, :],
                    OP.mult,
                )

            def flush_pend():
                if pend[0] is None:
                    return
                for g in range(NG):
                    emit_ctx_one(g)
                emit_norm()

            def attend_head(h, ic, extra=None):
                """S matmuls + exp-converts for head h; ctx+norm for pending."""
                isl = slice(ic * 512, (ic + 1) * 512)
                hb = 32 * (h % 4)
                hs = h // 4
                tp = (hb, 0)
                ct = cpool.tile([P, 512], F32, tag="ct", name="ct")
                pts = []
                for g in range(NG):
                    pt = ppool.tile([P, 2, 512], I8, tag="pt")
                    ps2 = spool.tile([P, 2, 512], F32, tag="s", name="s")
                    for half in range(2):
                        jt = 2 * g + half
                        jsl = slice(jt * P, (jt + 1) * P)
                        nc.tensor.matmul(
                            ps2[:, half, :],
                            khT[hb : hb + 32, hs, :, jsl],
                            qhT[hb : hb + 32, hs, :, isl],
                            start=True,
                            stop=True,
                            perf_mode=PM.DoubleRow,
                            tile_position=tp,
                        )
                    s_convert(pt, ps2)
                    pts.append(pt)
                    if pend[0] is not None:
                        emit_ctx_one(g)
                    if extra is not None and g in (3, 6):
                        extra(g)
                if pend[0] is not None:
                    emit_norm()
                pend[0] = (h, ic, pts, ct)

            def out_block(tt):
                """Out-proj + residual + LayerNorm for tokens [tt*128,...)."""
                tsl = slice(tt * P, (tt + 1) * P)
                res = opool.tile([P, D], F32, tag="res", name="res")
                nc.sync.dma_start(res, q_d[tsl, :])
                o32 = opool.tile([P, D], F32, tag="o32")
                for oc in range(2):
                    osl = slice(oc * 512, (oc + 1) * 512)
                    po = spool.tile([P, 512], F32, tag="s", name="po")
                    for pr in range(4):
                        nc.tensor.matmul(
                            po,
                            ctx8[:, pr, :, tsl],
                            wo[:, pr, :, osl],
                            start=(pr == 0),
                            stop=(pr == 3),
                            perf_mode=PM.DoubleRow,
                        )
                    nc.gpsimd.scalar_tensor_tensor(
                        o32[:, osl], po, 1.0 / 512.0, res[:, osl],
                        OP.mult, OP.add,
                    )
                st = npool.tile([P, 2, 6], F32, tag="st")
                nc.vector.bn_stats(st[:, 0, :], o32[:, 0:512])
                nc.vector.bn_stats(st[:, 1, :], o32[:, 512:1024])
                mv = npool.tile([P, 2], F32, tag="mv")
                nc.vector.bn_aggr(mv, st)
                veps = npool.tile([P, 1], F32, tag="veps")
                nc.vector.tensor_scalar_add(veps, mv[:, 1:2], EPS)
                sq = npool.tile([P, 1], F32, tag="sq")
                nc.scalar.activation(sq, veps, AF.Sqrt)
                rstd = npool.tile([P, 1], F32, tag="rstd")
                nc.vector.reciprocal(rstd, sq)
                # (x - mu) * rstd == x * rstd + (-mu * rstd), on ACT
                nmr = npool.tile([P, 1], F32, tag="nmr")
                nc.vector.tensor_scalar(
                    nmr, mv[:, 0:1], rstd, -1.0, OP.mult, OP.mult
                )
                xn = opool.tile([P, D], F32, tag="xn")
                nc.scalar.activation(
                    xn, o32, AF.Identity, bias=nmr[:, 0:1], scale=rstd[:, 0:1]
                )
                nc.sync.dma_start(out_d[tsl, :], xn)

            # ic=0 attention, with Q chunk 1 projection interleaved
            qproj_state = [0]

            def q1_extra(g):
                m = qproj_state[0]
                if m < 8:
                    qproj_state[0] += 1
                    csl = slice(512, 1024)
                    ps = proj_psum(wq, qT, m, csl)
                    conv_copy(qhT[:, m // 2, m % 2, csl], ps)

            for h in range(H):
                attend_head(h, 0, extra=q1_extra if h < 4 else None)
            # ic=1 attention with ic=0 out-blocks interleaved
            ob_state = [0]

            def ob_extra(g):
                tt = ob_state[0]
                if tt < 4 and g == 3:
                    ob_state[0] += 1
                    out_block(tt)

            for h in range(H):
                attend_head(h, 1, extra=ob_extra if h % 4 == 3 else None)
            flush_pend()
            for tt in range(ob_state[0], 8):
                out_block(tt)

    nc.compile()
    return nc


# ---------------------------------------------------------------------------
# fallback: original bf16 kernel (handles mask + gamma/beta)
# ---------------------------------------------------------------------------


def _build_fallback(masked, nogb):
    nc = bacc.Bacc(None, target_bir_lowering=False)

    q_d = nc.declare_dram_parameter("q", [TQ, D], F32, isOutput=False)
    qT_d = nc.declare_dram_parameter("qT", [D, TQ], BF, isOutput=False)
    kT_d = nc.declare_dram_parameter("kT", [D, SEQ], BF, isOutput=False)
    vT_d = nc.declare_dram_parameter("vT", [D, SEQ], BF, isOutput=False)
    m_d = nc.declare_dram_parameter("mask", [1, SEQ], F32, isOutput=False)
    wqT_d = nc.declare_dram_parameter("wqT", [D, D], BF, isOutput=False)
    wkT_d = nc.declare_dram_parameter("wkT", [D, D], BF, isOutput=False)
    wvT_d = nc.declare_dram_parameter("wvT", [D, D], BF, isOutput=False)
    woT_d = nc.declare_dram_parameter("woT", [D, D], BF, isOutput=False)
    g_d = nc.declare_dram_parameter("gamma", [1, D], F32, isOutput=False)
    b_d = nc.declare_dram_parameter("beta", [1, D], F32, isOutput=False)
    out_d = nc.declare_dram_parameter("out", [TQ, D], F32, isOutput=True)

    NPAIR = H // 2
    _sring = [None]
    with tile.TileContext(nc) as tc:
        with (
            tc.tile_pool(name="consts", bufs=1) as consts,
            tc.tile_pool(name="persist", bufs=1) as persist,
            tc.tile_pool(name="wvo", bufs=1) as wvo,
        ):
            if masked:
                msk = consts.tile([P, NJT], F32)
                with nc.allow_non_contiguous_dma(reason="tiny mask transpose"):
                    nc.sync.dma_start(msk, m_d[0].rearrange("(jt p) -> p jt", p=P))
                nc.vector.tensor_scalar_mul(msk, msk, NEG)

            qhT = persist.tile([P, NPAIR, TQ], BF, tag="qhT")
            khT = persist.tile([P, NPAIR, SEQ], BF, tag="khT")
            vh = persist.tile([P, NJT, H, DH + 1], BF, tag="vh")
            ctx = persist.tile([P, NPAIR, TQ], BF, tag="ctx")

            wvT = wvo.tile([P, 8, D], BF, tag="wvT")
            woT = wvo.tile([P, 8, D], BF, tag="woT")
            wkT = wvo.tile([P, 8, D], BF, tag="wkT")

            def load_wT(wT_dram, dst):
                for dk in range(8):
                    nc.sync.dma_start(dst[:, dk, :], wT_dram[dk * P : (dk + 1) * P, :])

            def load_xT(xT_dram, ch, dst):
                csl = slice(ch * 512, (ch + 1) * 512)
                for dk in range(8):
                    nc.sync.dma_start(dst[:, dk, :], xT_dram[dk * P : (dk + 1) * P, csl])

            def v_chunk(ch, xt_pool, pp_pool):
                vT = xt_pool.tile([P, 8, 512], BF, tag="xT")
                load_xT(vT_d, ch, vT)
                for ts_ in range(4):
                    jt = ch * 4 + ts_
                    for oc in range(2):
                        if pp_pool is None:
                            ps2 = _sring[0].tile([P, 2, 512], F32, tag="s", name="vps")
                            ps = ps2[:, 0, :]
                        else:
                            ps = pp_pool.tile([P, 512], F32, tag="pp")
                        for dk in range(8):
                            nc.tensor.matmul(
                                ps,
                                vT[:, dk, ts_ * P : (ts_ + 1) * P],
                                wvT[:, dk, oc * 512 : (oc + 1) * 512],
                                start=(dk == 0),
                                stop=(dk == 7),
                            )
                        nc.vector.tensor_copy(
                            vh[:, jt, oc * 8 : (oc + 1) * 8, 0:DH],
                            ps.rearrange("p (h c) -> p h c", c=DH),
                        )

            def k_chunk(ch, xt_pool, pp_pool):
                xT = xt_pool.tile([P, 8, 512], BF, tag="xT")
                load_xT(kT_d, ch, xT)
                csl = slice(ch * 512, (ch + 1) * 512)
                for ot in range(8):
                    if pp_pool is None:
                        ps2 = _sring[0].tile([P, 2, 512], F32, tag="s", name="kps")
                        ps = ps2[:, 0, :]
                    else:
                        ps = pp_pool.tile([P, 512], F32, tag="pp")
                    for dk in range(8):
                        nc.tensor.matmul(
                            ps,
                            wkT[:, dk, ot * P : (ot + 1) * P],
                            xT[:, dk, :],
                            start=(dk == 0),
                            stop=(dk == 7),
                        )
                    nc.vector.tensor_copy(khT[:, ot, csl], ps)

            def proj_qk(xT_dram, wT_dram, dst, nch, wpool, xt_pool, pp):
                wT_ = wpool.tile([P, 8, D], BF, tag="wT")
                for ch in range(nch):
                    xT = xt_pool.tile([P, 8, 512], BF, tag="xT")
                    if ch == 0:
                        for dk in range(8):
                            nc.sync.dma_start(
                                wT_[:, dk, :], wT_dram[dk * P : (dk + 1) * P, :]
                            )
                            nc.sync.dma_start(
                                xT[:, dk, :], xT_dram[dk * P : (dk + 1) * P, 0:512]
                            )
                    else:
                        load_xT(xT_dram, ch, xT)
                    for ot in range(8):
                        ps = pp.tile([P, 512], F32, tag="pp")
                        for dk in range(8):
                            nc.tensor.matmul(
                                ps,
                                wT_[:, dk, ot * P : (ot + 1) * P],
                                xT[:, dk, :],
                                start=(dk == 0),
                                stop=(dk == 7),
                            )
                        nc.vector.tensor_copy(
                            dst[:, ot, ch * 512 : (ch + 1) * 512], ps
                        )

            with (
                tc.tile_pool(name="xt1", bufs=3) as xt1,
                tc.tile_pool(name="pp1", bufs=3, space="PSUM") as pp1,
            ):
                with tc.tile_pool(name="wq", bufs=1) as wq_p:
                    proj_qk(qT_d, wqT_d, qhT, 2, wq_p, xt1, pp1)
                load_wT(wkT_d, wkT)
                load_wT(wvT_d, wvT)
                k_chunk(0, xt1, pp1)
                nc.vector.memset(vh[:, :, :, DH : DH + 1], 1.0)
                v_chunk(0, xt1, pp1)

            with (
                tc.tile_pool(name="xt2", bufs=2) as xt2,
                tc.tile_pool(name="ppool", bufs=3) as ppool,
                tc.tile_pool(name="bcp", bufs=2) as bcp,
                tc.tile_pool(name="ps_s", bufs=2, space="PSUM") as ps_s,
                tc.tile_pool(name="ps_ctx", bufs=2, space="PSUM") as ps_ctx,
            ):
                _sring[0] = ps_s

                def emit_ctx(ct2, pp, pA_, pB_, g_):
                    for half in range(2):
                        jt = 2 * g_ + half
                        nc.tensor.matmul(
                            ct2[0:65, 0, :], vh[:, jt, 2 * pp, :], pA_[:, half, :],
                            start=(jt == 0), stop=(jt == NJT - 1),
                        )
                        nc.tensor.matmul(
                            ct2[0:65, 1, :], vh[:, jt, 2 * pp + 1, :],
                            pB_[:, half, :],
                            start=(jt == 0), stop=(jt == NJT - 1),
                        )

                def attend_tail(pp, ic, ct2, prev):
                    isl = slice(ic * 512, (ic + 1) * 512)
                    emit_ctx(ct2, pp, *prev)
                    ctA = ct2[:, 0, :]
                    ctB = ct2[:, 1, :]
                    rrA = bcp.tile([P, 512], F32, tag="rrow")
                    rrB = bcp.tile([P, 512], F32, tag="rrow")
                    nc.vector.reciprocal(rrA[0:1, :], ctA[64:65, :])
                    nc.vector.reciprocal(rrB[0:1, :], ctB[64:65, :])
                    bcA = bcp.tile([P, 512], F32, tag="bc")
                    bcB = bcp.tile([P, 512], F32, tag="bc")
                    nc.gpsimd.partition_broadcast(bcA, rrA[0:1, :])
                    nc.gpsimd.partition_broadcast(bcB, rrB[0:1, :])
                    nc.vector.tensor_mul(
                        ctx[0:64, pp, isl], ctA[0:64, :], bcA[0:64, :]
                    )
                    nc.vector.tensor_mul(
                        ctx[64:128, pp, isl], ctB[0:64, :], bcB[0:64, :]
                    )

                pending = [None]

                def attend(pp, ic, after_grp=None):
                    isl = slice(ic * 512, (ic + 1) * 512)
                    ct2 = ps_ctx.tile([P, 2, 512], F32, tag="ctx2")

                    prev = None
                    for g in range(NG):
                        sA = ps_s.tile([P, 2, 512], F32, tag="s")
                        sB = ps_s.tile([P, 2, 512], F32, tag="s")
                        for half in range(2):
                            jsl = slice((2 * g + half) * P, (2 * g + half + 1) * P)
                            nc.tensor.matmul(
                                sA[:, half, :],
                                khT[0:64, pp, jsl], qhT[0:64, pp, isl],
                                start=True, stop=True,
                            )
                        for half in range(2):
                            jsl = slice((2 * g + half) * P, (2 * g + half + 1) * P)
                            nc.tensor.matmul(
                                sB[:, half, :],
                                khT[64:128, pp, jsl], qhT[64:128, pp, isl],
                                start=True, stop=True,
                            )
                        pA = ppool.tile([P, 2, 512], BF, tag="pA")
                        pB = ppool.tile([P, 2, 512], BF, tag="pB")
                        if masked:
                            for half in range(2):
                                jt = 2 * g + half
                                bias = msk[:, jt : jt + 1]
                                nc.scalar.activation(
                                    pA[:, half, :], sA[:, half, :], AF.Exp,
                                    bias=bias, scale=0.125,
                                )
                                nc.scalar.activation(
                                    pB[:, half, :], sB[:, half, :], AF.Exp,
                                    bias=bias, scale=0.125,
                                )
                        else:
                            nc.scalar.activation(pA, sA, AF.Exp, scale=0.125)
                            nc.scalar.activation(pB, sB, AF.Exp, scale=0.125)
                        if prev is not None:
                            emit_ctx(ct2, pp, *prev)
                        prev = (pA, pB, g)
                        if g == 0 and pending[0] is not None:
                            attend_tail(*pending[0])
                            pending[0] = None
                        if after_grp is not None:
                            after_grp(g)
                    pending[0] = (pp, ic, ct2, prev)

                def v_inline(g):
                    if g in (1, 3, 5):
                        k_chunk(1 + g // 2, xt2, None)
                        v_chunk(1 + g // 2, xt2, None)

                attend(0, 0, after_grp=v_inline)
                attend(0, 1)
                load_wT(woT_d, woT)
                for pp in range(1, NPAIR):
                    for ic in range(NIC):
                        attend(pp, ic)
                attend_tail(*pending[0])

            with (
                tc.tile_pool(name="lnc", bufs=1) as lnc,
                tc.tile_pool(name="res", bufs=2) as resp,
                tc.tile_pool(name="outp", bufs=2) as outp,
                tc.tile_pool(name="stat", bufs=2) as stat,
                tc.tile_pool(name="ps_o", bufs=4, space="PSUM") as ps_o,
            ):
                if not nogb:
                    gam = lnc.tile([P, D], F32)
                    bet = lnc.tile([P, D], F32)
                    nc.sync.dma_start(gam, bcast_pap(g_d[0]))
                    nc.sync.dma_start(bet, bcast_pap(b_d[0]))
                for tt in range(TQ // P):
                    tsl = slice(tt * P, (tt + 1) * P)
                    res = resp.tile([P, D], F32, tag="res")
                    nc.sync.dma_start(res, q_d[tsl, :])
                    o32 = outp.tile([P, D], F32, tag="o32")
                    for oc in range(2):
                        osl = slice(oc * 512, (oc + 1) * 512)
                        ps = ps_o.tile([P, 512], F32, tag="po")
                        for kt in range(8):
                            nc.tensor.matmul(
                                ps, ctx[:, kt, tsl], woT[:, kt, osl],
                                start=(kt == 0), stop=(kt == 7),
                            )
                        nc.vector.tensor_add(o32[:, osl], ps, res[:, osl])
                    st = stat.tile([P, 2, 6], F32, tag="st")
                    nc.vector.bn_stats(st[:, 0, :], o32[:, 0:512])
                    nc.vector.bn_stats(st[:, 1, :], o32[:, 512:1024])
                    mv = stat.tile([P, 2], F32, tag="mv")
                    nc.vector.bn_aggr(mv, st)
                    veps = stat.tile([P, 1], F32, tag="veps")
                    nc.vector.tensor_scalar_add(veps, mv[:, 1:2], EPS)
                    sq = stat.tile([P, 1], F32, tag="sq")
                    nc.scalar.activation(sq, veps, AF.Sqrt)
                    rstd = stat.tile([P, 1], F32, tag="rstd")
                    nc.vector.reciprocal(rstd, sq)
                    xn = outp.tile([P, D], F32, tag="xn")
                    nc.vector.tensor_scalar(
                        xn, o32, mv[:, 0:1], rstd, OP.subtract, OP.mult
                    )
                    if not nogb:
                        nc.vector.tensor_mul(xn, xn, gam)
                        nc.vector.tensor_add(xn, xn, bet)
                    nc.sync.dma_start(out_d[tsl, :], xn)

    nc.compile()
    return nc


_NC = {}


def _get_nc(masked=False, nogb=True):
    key = (masked, nogb)
    if key not in _NC:
        if not masked and nogb:
            _NC[key] = _build_fp8()
        else:
            _NC[key] = _build_fallback(masked, nogb)
    return _NC[key]


def _dr_layout(xT):
    """[d, n] -> [128, 4, 2, n] DoubleRow layout: d = 128*(2*pr+i) + p."""
    d, n = xT.shape
    return np.ascontiguousarray(xT.reshape(4, 2, P, n).transpose(2, 0, 1, 3))


_QK_PERM = np.empty(D, dtype=np.int64)
for _m in range(8):
    for _p in range(P):
        _QK_PERM[_m * P + _p] = (
            64 * (4 * (_m // 2) + _p // 32) + 32 * (_m % 2) + _p % 32
        )

_CTX_ROW = np.empty(D, dtype=np.int64)  # wo row for ctx8 channel layout
for _pr in range(4):
    for _i in range(2):
        for _p in range(P):
            ch = 128 * (2 * _pr + _i) + _p  # = 64*h + c with h=2*kt+(p>=64)
            _CTX_ROW[128 * (2 * _pr + _i) + _p] = ch


def kernel(q, k, v, zero_mask, Wq, Wk, Wv, Wo, gamma, beta):
    q = np.ascontiguousarray(np.asarray(q, dtype=np.float32))
    k = np.ascontiguousarray(np.asarray(k, dtype=np.float32))
    v = np.ascontiguousarray(np.asarray(v, dtype=np.float32))
    zero_mask = np.ascontiguousarray(np.asarray(zero_mask, dtype=np.float32))
    gamma_a = np.asarray(gamma, dtype=np.float32).reshape(-1)
    beta_a = np.asarray(beta, dtype=np.float32).reshape(-1)

    masked = bool(np.any(zero_mask != 0.0))
    nogb = bool(np.all(gamma_a == 1.0) and np.all(beta_a == 0.0))
    nc = _get_nc(masked=masked, nogb=nogb)

    if not masked and nogb:
        sw = np.float32(W_SIDE_SCALE)
        wq8 = _dr_layout((Wq * sw).T[:, _QK_PERM].astype(E4_NP))
        wk8 = _dr_layout((Wk * sw).T[:, _QK_PERM].astype(E4_NP))
        wv8 = _dr_layout((Wv * 16.0).T.astype(E4_NP))
        wo8 = _dr_layout(np.ascontiguousarray((Wo * 32.0).T)[_CTX_ROW, :].astype(E4_NP))
        kT8 = [_dr_layout(k[b].T.astype(E4_NP)) for b in range(BS)]
        vT8 = [_dr_layout(v[b].T.astype(E4_NP)) for b in range(BS)]
        in_maps = []
        for c in range(NCORE):
            b, hf = c // 2, c % 2
            qs = q[b, hf * TQ : (hf + 1) * TQ, :]
            in_maps.append(
                {
                    "qT8": _dr_layout(qs.T.astype(E4_NP)),
                    "kT8": kT8[b],
                    "vT8": vT8[b],
                    "wq8": wq8,
                    "wk8": wk8,
                    "wv8": wv8,
                    "wo8": wo8,
                    "q": np.ascontiguousarray(qs),
                }
            )
    else:
        gamma2 = gamma_a.reshape(1, D)
        beta2 = beta_a.reshape(1, D)
        wT = {
            n: np.ascontiguousarray(np.asarray(w, dtype=np.float32).T.astype(BF_NP))
            for n, w in (("wqT", Wq), ("wkT", Wk), ("wvT", Wv), ("woT", Wo))
        }
        kT = [np.ascontiguousarray(k[b].T.astype(BF_NP)) for b in range(BS)]
        vT = [np.ascontiguousarray(v[b].T.astype(BF_NP)) for b in range(BS)]
        in_maps = []
        for c in range(NCORE):
            b, hf = c // 2, c % 2
            qs = q[b, hf * TQ : (hf + 1) * TQ, :]
            in_maps.append(
                {
                    "q": np.ascontiguousarray(qs),
                    "qT": np.ascontiguousarray(qs.T.astype(BF_NP)),
                    "kT": kT[b],
                    "vT": vT[b],
                    "mask": np.ascontiguousarray(zero_mask[b, 0]),
                    "gamma": gamma2,
                    "beta": beta2,
                    **wT,
                }
            )

    res = run_bass_kernel_spmd(nc, in_maps, list(range(NCORE)))
    out = np.empty((BS, SEQ, D), dtype=np.float32)
    for c in range(NCORE):
        b, hf = c // 2, c % 2
        out[b, hf * TQ : (hf + 1) * TQ, :] = res.results[c]["out"]
    return out
